# revision 9
# baseline (speedup 1.0000x reference)
"""Trainium2 Bass kernel for a 4-layer decoder LSTM with Luong attention. v2

Strategy (8 NeuronCores, SPMD — identical program, per-core data):
  - LSTM weights tensor-parallel sharded over the gate dim: core c owns
    h-dims [128c, 128c+128) of every layer (512 of the 4096 gate rows).
    Per layer/step each core computes its gate slice, then the h.T slices
    are AllGathered (partition-axis concat == feature concat).
  - Attention sharded over batch: core c owns batches [8c, 8c+8).
    All on-chip: all-pairs scores -> SEL matmul -> masked-reduce diagonal
    -> softmax -> masked dup + PE transpose to build the block-diagonal
    attention operand (no DRAM roundtrips).
  - Vocab projection sharded over V (4000 rows/core, padded to 4096).
    Projection for group g is computed during the steps of group g+1,
    its matmuls interleaved into the collective-latency gaps; w_proj
    tiles streamed from HBM; outputs written in a partition-contiguous
    DRAM layout and transposed on host.
All recurrence matmuls run in bf16 with fp32 PSUM accumulation; the cell
state c stays fp32.
"""

import os
import numpy as np
from contextlib import ExitStack

import concourse.bass as bass
import concourse.bacc as bacc
import concourse.mybir as mybir
import concourse.tile as tile
from concourse.bass_utils import run_bass_kernel_spmd
from concourse.masks import make_identity

V, E, H, L, ENC = 32000, 512, 1024, 4, 1024
B, T, S = 64, 33, 64
TS = T - 1                      # 32 decode steps
NCORES = 8
BSH = B // NCORES               # 8 batches / core (attention shard)
HSH = H // NCORES               # 128 h-dims / core
GSH = 4 * HSH                   # 512 gate rows / core
VSH = V // NCORES               # 4000 vocab rows / core
VP = 4096                       # padded vocab shard
GRP = 4                         # steps per projection group
NM = VP // 128                  # 32 m-tiles of the vocab shard
P = 128

FP32 = mybir.dt.float32
BF16 = mybir.dt.bfloat16
I32 = mybir.dt.int32
DT = BF16
ALL = [list(range(NCORES))]

_prog_cache = {}

# Packed-input segment tables: all fp32 inputs ride one flat array, all bf16
# inputs another (fewer buffer handles per execute). Order here defines the
# host-side concat order in _prep_inputs.
F32_SEGS = [
    ("h03", (P, 8, B)), ("c0s", (P, L, B)), ("encT", (P, 8, BSH * S)),
    ("wattn", (P, 8, H)), ("bias", (P, L * 4)), ("sel", (B, BSH)),
    ("msk", (BSH, BSH * S)), ("mev", (BSH, 2)), ("bp", (P, NM)),
]
BF_SEGS = [
    ("h0T", (P, L, 8, B)), ("encs", (P, 4, ENC)), ("wl0", (P, 20, GSH)),
    ("wl", (P, L - 1, 16, GSH)), ("wp", (NM, P, 16, P)),
]


def _seg_offsets(segs):
    offs, tot = {}, 0
    for name, shp in segs:
        n = int(np.prod(shp))
        offs[name] = (tot, n, shp)
        tot += n
    return offs, tot


F32_OFF, F32_TOT = _seg_offsets(F32_SEGS)
BF_OFF, BF_TOT = _seg_offsets(BF_SEGS)


def _build(nsteps: int):
    """Build the SPMD Bass program (same for every core)."""
    nc = bacc.Bacc("TRN2", num_devices=NCORES)

    # ---- external inputs (host packs everything in SBUF layout, partition-major)
    d_idx = nc.dram_tensor("d_idx", [P, TS * B // P], I32, kind="ExternalInput")
    d_emb = nc.dram_tensor("d_emb", [V, E], FP32, kind="ExternalInput")
    d_pf32 = nc.dram_tensor("d_pf32", [F32_TOT], FP32, kind="ExternalInput")
    d_pbf = nc.dram_tensor("d_pbf", [BF_TOT], DT, kind="ExternalInput")
    d_out = nc.dram_tensor("d_out", [NM, P, TS, B], BF16, kind="ExternalOutput")

    def fseg(name):
        off, n, shp = F32_OFF[name]
        ap = d_pf32[off:off + n]
        pat = "(" + " ".join(f"a{i}" for i in range(len(shp))) + ") -> " + \
              " ".join(f"a{i}" for i in range(len(shp)))
        return ap.rearrange(pat, **{f"a{i}": s for i, s in enumerate(shp)})

    def bseg(name):
        off, n, shp = BF_OFF[name]
        ap = d_pbf[off:off + n]
        pat = "(" + " ".join(f"a{i}" for i in range(len(shp))) + ") -> " + \
              " ".join(f"a{i}" for i in range(len(shp)))
        return ap.rearrange(pat, **{f"a{i}": s for i, s in enumerate(shp)})

    d_h0T, d_encs, d_wl0, d_wl = (bseg(n) for n in ("h0T", "encs", "wl0", "wl"))
    d_wp = bseg("wp")
    d_h03, d_c0s, d_encT, d_wattn = (fseg(n) for n in
                                     ("h03", "c0s", "encT", "wattn"))
    d_bias, d_sel, d_msk, d_mev, d_bp = (fseg(n) for n in
                                         ("bias", "sel", "msk", "mev", "bp"))

    with tile.TileContext(nc) as tc, ExitStack() as ctx:
        cw = ctx.enter_context(tc.tile_pool(name="cw", bufs=1))
        st = ctx.enter_context(tc.tile_pool(name="st", bufs=1))
        sb = ctx.enter_context(tc.tile_pool(name="sb", bufs=3))
        ps = ctx.enter_context(tc.tile_pool(name="ps", bufs=2, space="PSUM"))
        gtp = ctx.enter_context(tc.tile_pool(name="gtp", bufs=1, space="PSUM"))
        prp = ctx.enter_context(tc.tile_pool(name="prp", bufs=2, space="PSUM"))
        apool = ctx.enter_context(tc.tile_pool(name="apool", bufs=2))
        wstr = ctx.enter_context(tc.tile_pool(name="wstr", bufs=3))
        dr = ctx.enter_context(tc.tile_pool(name="dr", bufs=3, space="DRAM"))
        drs = ctx.enter_context(tc.tile_pool(name="drs", bufs=3, space="DRAM"))

        # ---- persistent SBUF residents
        WL0 = cw.tile([P, 20, GSH], DT)
        nc.sync.dma_start(WL0[:], d_wl0[:])
        WL = cw.tile([P, L - 1, 16, GSH], DT)
        nc.sync.dma_start(WL[:], d_wl[:])
        ET = cw.tile([P, 8, BSH * S], FP32)
        nc.sync.dma_start(ET[:], d_encT[:])
        ES = cw.tile([P, 4, ENC], DT)
        nc.sync.dma_start(ES[:], d_encs[:])
        BIA = cw.tile([P, L * 4], FP32)
        nc.sync.dma_start(BIA[:], d_bias[:])
        BP = cw.tile([P, NM], FP32)
        nc.sync.dma_start(BP[:], d_bp[:])
        IDX = cw.tile([P, TS * B // P], I32)
        nc.sync.dma_start(IDX[:], d_idx[:])
        SEL = cw.tile([B, BSH], FP32)
        nc.sync.dma_start(SEL[:], d_sel[:])
        MSK = cw.tile([BSH, BSH * S], FP32)
        nc.sync.dma_start(MSK[:], d_msk[:])
        MEV = cw.tile([BSH, 2], FP32)
        nc.sync.dma_start(MEV[:], d_mev[:])
        IDN = cw.tile([P, P], FP32)
        make_identity(nc, IDN[:])
        XT = cw.tile([P, 4, TS * B], DT)    # x embeddings, feature-major
        AT = cw.tile([P, 8, BSH * S], FP32)  # (enc @ w_attn).T shard

        # state tiles
        hT = [st.tile([P, 8, B], DT, name=f"hT{l}") for l in range(L)]
        hT3f = st.tile([P, 8, B], FP32)
        cS = [st.tile([P, B], FP32, name=f"cS{l}") for l in range(L)]
        for l in range(L):
            nc.sync.dma_start(hT[l][:], d_h0T[:, l])
            nc.sync.dma_start(cS[l][:], d_c0s[:, l])
        nc.sync.dma_start(hT3f[:], d_h03[:])

        # ---- parallel phase: embedding gather + transpose into XT
        for j in range(TS * B // P):
            xg = sb.tile([P, E], FP32, tag="xg")
            nc.gpsimd.indirect_dma_start(
                out=xg[:], out_offset=None, in_=d_emb[:],
                in_offset=bass.IndirectOffsetOnAxis(ap=IDX[:, j:j + 1], axis=0),
            )
            for e in range(4):
                tp = ps.tile([P, 512], FP32, tag="att", bufs=1)
                nc.tensor.transpose(tp[:, :P], xg[:, e * P:(e + 1) * P], IDN[:])
                nc.vector.tensor_copy(XT[:, e, j * P:(j + 1) * P], tp[:, :P])

        # ---- parallel phase: A.T shard = w_attn.T @ encT_shard
        for m in range(8):
            pa = ps.tile([P, BSH * S], FP32, tag="att", bufs=1)
            for k in range(8):
                wak = sb.tile([P, P], FP32, tag="wak")
                nc.sync.dma_start(wak[:], d_wattn[:, k, m * P:(m + 1) * P])
                nc.tensor.matmul(pa[:], wak[:], ET[:, k, :],
                                 start=(k == 0), stop=(k == 7))
            nc.vector.tensor_copy(AT[:, m, :], pa[:])

        sig = mybir.ActivationFunctionType.Sigmoid
        tanh = mybir.ActivationFunctionType.Tanh
        expf = mybir.ActivationFunctionType.Exp

        # projection state: group `pg` (steps [4pg, 4pg+4)) is computed
        # lazily during the steps of the next group, `done[m]` m-tiles at a
        # time. Schedule of m-tiles issued at each gap of a step:
        acts_cur = None     # acts buffer being filled this group
        acts_prev = None    # completed acts buffer of the previous group
        prev_t0 = 0
        prev_ngt = 0

        def project_mtiles(mlist, ngt, t0, acts_buf):
            """Issue projection matmuls for m-tiles of the previous group.
            All projection DMAs ride the Activation HWDGE queue to keep the
            SP queue free for the recurrence-critical transfers."""
            for m in mlist:
                wpt = wstr.tile([P, 16, P], DT, tag="wpt")
                nc.scalar.dma_start(wpt[:], d_wp[m])
                PR = prp.tile([P, GRP * B], FP32, tag="PR")
                for k in range(16):
                    nc.tensor.matmul(PR[:, :ngt * B], wpt[:, k, :],
                                     acts_buf[:, k, :ngt, :],
                                     start=(k == 0), stop=(k == 15))
                ev = sb.tile([P, GRP, B], BF16, tag="ev")
                nc.vector.tensor_scalar_add(ev[:, :ngt, :], PR[:, :ngt * B],
                                            BP[:, m:m + 1])
                nc.scalar.dma_start(d_out[m, :, t0:t0 + ngt, :], ev[:, :ngt, :])

        prev_issued = 0
        for t in range(nsteps):
            tg = t % GRP
            if tg == 0:
                acts_prev, acts_cur = acts_cur, apool.tile(
                    [P, 16, GRP, B], DT, tag="acts")
                prev_issued = 0
            # distribute the previous group's 32 m-tiles over this group's
            # steps: 8 tiles/step spread across the 5 collective-latency gaps
            gaps = [[], [], [], [], []]
            if acts_prev is not None:
                for gi, ng in enumerate([2, 2, 2, 1, 1]):
                    gaps[gi] = list(range(prev_issued, prev_issued + ng))
                    prev_issued += ng

            # ======== attention scores first (chain head), then the
            # independent partial gates fill the PE while DVE/Act run the
            # softmax ========
            Pp = ps.tile([B, BSH * S], FP32, tag="att", bufs=1)
            for k in range(8):
                nc.tensor.matmul(Pp[:], hT3f[:, k, :], AT[:, k, :],
                                 start=(k == 0), stop=(k == 7))
            Psb = sb.tile([B, BSH * S], FP32, tag="Psb")
            nc.vector.tensor_copy(Psb[:], Pp[:])

            # ======== independent partial gates (h(t-1), x(t)) ========
            # One open accumulation group per layer; each layer occupies its
            # own 2KB PSUM zero region ([P, 512] f32) so the four concurrently
            # open groups never share a region (start=True marks the whole
            # region pending-zero).
            GT4 = gtp.tile([P, L, 512], FP32, tag="GT4")
            for l in range(L):
                W = WL0 if l == 0 else WL[:, l - 1]
                # k-blocks 8..15 are W_hh (h_l(t-1)); for l=0 also x blocks
                rhs_ind = [(8 + k, hT[l][:, k, :]) for k in range(8)]
                if l == 0:
                    rhs_ind += [(16 + e, XT[:, e, t * B:(t + 1) * B])
                                for e in range(4)]
                for g in range(4):
                    for i, (kb, rhs) in enumerate(rhs_ind):
                        nc.tensor.matmul(GT4[:, l, g * B:(g + 1) * B],
                                         W[:, kb, g * P:(g + 1) * P], rhs,
                                         start=(g == 0 and i == 0), stop=False,
                                         skip_group_check=True)
            # select own 8 batch rows (core-id baked into SEL data);
            # Po reuses the att slot (Pp is dead once Psb is copied)
            Po = ps.tile([BSH, BSH * S], FP32, tag="att", bufs=1)
            nc.tensor.matmul(Po[:], SEL[:], Psb[:], start=True, stop=True)
            # masked reduce -> diagonal scores sc[j, s]
            Pm = sb.tile([BSH, BSH * S], FP32, tag="Pm")
            nc.vector.tensor_mul(Pm[:], Po[:], MSK[:])
            sc = sb.tile([BSH, S], FP32, tag="sc")
            nc.vector.tensor_reduce(
                sc[:], Pm[:].rearrange("j (jp s) -> j s jp", s=S),
                axis=mybir.AxisListType.X, op=mybir.AluOpType.add)
            # softmax over S
            mx = sb.tile([BSH, 1], FP32, tag="mx")
            nc.vector.tensor_reduce(mx[:], sc[:], axis=mybir.AxisListType.X,
                                    op=mybir.AluOpType.max)
            mxn = sb.tile([BSH, 1], FP32, tag="mxn")
            nc.vector.tensor_scalar_mul(mxn[:], mx[:], -1.0)
            ex = sb.tile([BSH, S], FP32, tag="ex")
            sm = sb.tile([BSH, 1], FP32, tag="sm")
            nc.scalar.activation(ex[:], sc[:], expf, bias=mxn[:, 0:1],
                                 accum_out=sm[:, 0:1])
            rc = sb.tile([BSH, 1], FP32, tag="rc")
            nc.vector.reciprocal(rc[:], sm[:])
            rcm = sb.tile([BSH, 2], FP32, tag="rcm")
            nc.vector.tensor_scalar_mul(rcm[:], MEV[:], rc[:, 0:1])
            # masked duplicate: atn2[j, 64a + s] = atn[j, s] * (a == j % 2)
            atn2 = sb.tile([BSH, 2, S], FP32, tag="atn2")
            nc.vector.tensor_scalar_mul(atn2[:, 0, :], ex[:], rcm[:, 0:1])
            nc.vector.tensor_scalar_mul(atn2[:, 1, :], ex[:], rcm[:, 1:2])
            # PE transpose -> TA[(a, s), j]; block-diag ABD via 4 copies.
            # TA and Cp share one PSUM bank (serial use within the step).
            att2 = ps.tile([P, 72], FP32, tag="att2", bufs=1)
            TA = att2[:, 64:72]
            Cp = att2[:, 0:64]
            nc.tensor.transpose(TA, atn2[:, :, :], IDN[:BSH, :BSH])
            ABD = sb.tile([P, 4, BSH], DT, tag="ABD")
            nc.gpsimd.memset(ABD[:], 0.0)
            for kk in range(4):
                nc.vector.tensor_copy(ABD[:, kk, 2 * kk:2 * kk + 2],
                                      TA[:, 2 * kk:2 * kk + 2])
            # ctx.T own-block [ENC, 8]
            for m in range(8):
                for kk in range(4):
                    nc.tensor.matmul(Cp[:, m * BSH:(m + 1) * BSH],
                                     ES[:, kk, m * P:(m + 1) * P],
                                     ABD[:, kk, :],
                                     start=(kk == 0), stop=(kk == 3))
            Co = sb.tile([P, 8, BSH], DT, tag="Co")
            nc.vector.tensor_copy(Co[:], Cp[:])
            # AllGather ctx blocks; single readback straight into the acts
            # buffer ((q j) == global batch order), which doubles as ctxT
            agic = dr.tile([8, P, BSH], DT, tag="agic")
            nc.sync.dma_start(agic[:].rearrange("m p j -> p m j"), Co[:])
            agoc = drs.tile([NCORES, 8, P, BSH], DT, addr_space="Shared", tag="agoc")
            nc.gpsimd.collective_compute(
                "AllGather", mybir.AluOpType.bypass, replica_groups=ALL,
                ins=[agic[:]], outs=[agoc[:]],
            )
            for e in range(8):
                eng = nc.sync if e % 2 == 0 else nc.scalar
                eng.dma_start(acts_cur[:, 8 + e, tg, :],
                              agoc[:, e].rearrange("q p j -> p q j"))
            ctxT = acts_cur[:, 8:16, tg, :]

            # gap 0: projection m-tiles ride the ctx-AG latency
            project_mtiles(gaps[0], prev_ngt, prev_t0, acts_prev)

            # ======== LSTM layers: dependent parts ========
            for l in range(L):
                if l == 0:
                    W = WL0
                    rhs_dep = [(k, ctxT[:, k, :]) for k in range(8)]
                else:
                    W = WL[:, l - 1]
                    rhs_dep = [(k, hT[l - 1][:, k, :]) for k in range(8)]
                for g in range(4):
                    for i, (kb, rhs) in enumerate(rhs_dep):
                        nc.tensor.matmul(GT4[:, l, g * B:(g + 1) * B],
                                         W[:, kb, g * P:(g + 1) * P], rhs,
                                         start=False,
                                         stop=(g == 3 and i == 7),
                                         skip_group_check=True)
                GT = GT4[:, l]
                gi = sb.tile([P, B], FP32, tag="gi")
                gf = sb.tile([P, B], FP32, tag="gf")
                gg = sb.tile([P, B], FP32, tag="gg")
                go = sb.tile([P, B], FP32, tag="go")
                nc.scalar.activation(gi[:], GT[:, 0 * B:1 * B], sig, bias=BIA[:, 4 * l + 0:4 * l + 1])
                nc.scalar.activation(gf[:], GT[:, 1 * B:2 * B], sig, bias=BIA[:, 4 * l + 1:4 * l + 2])
                nc.scalar.activation(gg[:], GT[:, 2 * B:3 * B], tanh, bias=BIA[:, 4 * l + 2:4 * l + 3])
                nc.scalar.activation(go[:], GT[:, 3 * B:4 * B], sig, bias=BIA[:, 4 * l + 3:4 * l + 4])
                t1 = sb.tile([P, B], FP32, tag="t1")
                nc.vector.tensor_mul(t1[:], gf[:], cS[l][:])
                t2 = sb.tile([P, B], FP32, tag="t2")
                nc.vector.tensor_mul(t2[:], gi[:], gg[:])
                nc.vector.tensor_add(cS[l][:], t1[:], t2[:])
                th = sb.tile([P, B], FP32, tag="th")
                nc.scalar.activation(th[:], cS[l][:], tanh)
                hdt = FP32 if l == L - 1 else DT
                hsl = sb.tile([P, B], hdt, tag=f"hsl{l == L - 1}")
                nc.vector.tensor_mul(hsl[:], go[:], th[:])
                # AllGather h slices (layer 3 in fp32 for the attention scores)
                agih = dr.tile([P, B], hdt, tag=f"agih{l == L - 1}")
                nc.sync.dma_start(agih[:], hsl[:])
                agoh = drs.tile([NCORES, P, B], hdt, addr_space="Shared",
                                tag=f"agoh{l == L - 1}")
                nc.gpsimd.collective_compute(
                    "AllGather", mybir.AluOpType.bypass, replica_groups=ALL,
                    ins=[agih[:]], outs=[agoh[:]],
                )
                if l == L - 1:
                    nc.sync.dma_start(hT3f[:], agoh[:].rearrange("q p b -> p q b"))
                    nc.vector.tensor_copy(hT[l][:], hT3f[:])
                    nc.vector.tensor_copy(acts_cur[:, 0:8, tg, :], hT3f[:])
                else:
                    nc.sync.dma_start(hT[l][:], agoh[:].rearrange("q p b -> p q b"))
                # gap l+1: more projection m-tiles ride this AG's latency
                project_mtiles(gaps[l + 1], prev_ngt, prev_t0, acts_prev)

            if tg == GRP - 1:
                prev_t0, prev_ngt = t - 3, 4

        # tail: flush any remaining m-tiles of the previous group, then the
        # final (possibly partial) group's projection
        if acts_prev is not None and prev_issued < NM:
            project_mtiles(list(range(prev_issued, NM)), prev_ngt, prev_t0,
                           acts_prev)
        last_t0 = (nsteps - 1) // GRP * GRP
        last_ngt = nsteps - last_t0
        project_mtiles(list(range(NM)), last_ngt, last_t0, acts_cur)
    nc.finalize()
    return nc


def _prep_inputs(tgt, h0, c0, enc_out, embedding, w_attn, w_ih0, w_ih_rest,
                 w_hh, b_ih, b_hh, w_proj, b_proj):
    """Pack per-core input dicts (numpy only: slicing / layout / dtype prep)."""
    bf = mybir.dt.np(BF16)
    f32 = np.float32

    idx = np.ascontiguousarray(tgt[:, :TS].T.astype(np.int32).reshape(-1))  # t-major
    d_idx = idx.reshape(TS * B // P, P).T.copy()  # [P, chunks] partition-major

    # h0T: [P, L, 8, B]
    h0T = np.stack([h0[l].T.reshape(8, P, B) for l in range(L)], 0)  # [L, 8, P, B]
    d_h0T = np.ascontiguousarray(h0T.transpose(2, 0, 1, 3)).astype(bf)
    d_h03 = np.ascontiguousarray(h0T[L - 1].transpose(1, 0, 2)).astype(f32)

    bias = (b_ih + b_hh).astype(f32)  # [L, 4H]

    # diagonal-extraction mask [BSH, BSH*S]: 1 where j' == j
    d_msk = np.zeros((BSH, BSH * S), f32)
    for j in range(BSH):
        d_msk[j, j * S:(j + 1) * S] = 1.0
    # even/odd row masks [BSH, 2]
    d_mev = np.zeros((BSH, 2), f32)
    d_mev[0::2, 0] = 1.0
    d_mev[1::2, 1] = 1.0

    in_maps = []
    for c in range(NCORES):
        rows = np.concatenate([np.arange(g * H + c * HSH, g * H + (c + 1) * HSH)
                               for g in range(4)])  # gate rows, order (g, d)
        # c0 slice: [P, L, B]
        d_c0s = np.ascontiguousarray(
            np.stack([c0[l, :, c * HSH:(c + 1) * HSH].T for l in range(L)], 0)
            .transpose(1, 0, 2)).astype(f32)
        # enc shard
        encb = enc_out[c * BSH:(c + 1) * BSH]              # [8, S, ENC]
        encT = encb.transpose(2, 0, 1).reshape(ENC, BSH * S)  # [ENC, 512]
        d_encT = np.ascontiguousarray(encT.reshape(8, P, BSH * S)
                                      .transpose(1, 0, 2)).astype(f32)
        d_encs = np.ascontiguousarray(encb.reshape(4, P, ENC)
                                      .transpose(1, 0, 2)).astype(bf)
        d_wattn = np.ascontiguousarray(w_attn.reshape(8, P, H)
                                       .transpose(1, 0, 2)).astype(f32)
        # layer-0 weights: K order [ctx(8) | h0(8) | x(4)]
        w0 = np.concatenate([w_ih0[rows, E:], w_hh[0][rows], w_ih0[rows, :E]], 1)
        d_wl0 = np.ascontiguousarray(w0.T.reshape(20, P, GSH)
                                     .transpose(1, 0, 2)).astype(bf)
        wls = []
        for l in range(1, L):
            wcat = np.concatenate([w_ih_rest[l - 1][rows], w_hh[l][rows]], 1)
            wls.append(wcat.T.reshape(16, P, GSH))
        d_wl = np.ascontiguousarray(np.stack(wls, 0)
                                    .transpose(2, 0, 1, 3)).astype(bf)
        d_bias = np.ascontiguousarray(
            bias[:, rows].reshape(L, 4, P).transpose(2, 0, 1).reshape(P, L * 4)
        ).astype(f32)
        d_sel = np.zeros((B, BSH), f32)
        d_sel[np.arange(c * BSH, (c + 1) * BSH), np.arange(BSH)] = 1.0
        # w_proj shard, padded + tiled [m, p, k, col]
        wpad = np.zeros((VP, 2 * H), f32)
        wpad[:VSH] = w_proj[c * VSH:(c + 1) * VSH]
        d_wp = np.ascontiguousarray(wpad.T.reshape(16, P, NM, P)
                                    .transpose(2, 1, 0, 3)).astype(bf)
        bpad = np.zeros((VP,), f32)
        bpad[:VSH] = b_proj[c * VSH:(c + 1) * VSH]
        d_bp = np.ascontiguousarray(bpad.reshape(NM, P).T).astype(f32)

        fvals = {"h03": d_h03, "c0s": d_c0s, "encT": d_encT, "wattn": d_wattn,
                 "bias": d_bias, "sel": d_sel, "msk": d_msk, "mev": d_mev,
                 "bp": d_bp}
        bvals = {"h0T": d_h0T, "encs": d_encs, "wl0": d_wl0, "wl": d_wl,
                 "wp": d_wp}
        d_pf32 = np.concatenate([np.ascontiguousarray(fvals[n]).ravel()
                                 for n, _ in F32_SEGS])
        d_pbf = np.concatenate([np.ascontiguousarray(bvals[n]).ravel()
                                for n, _ in BF_SEGS])
        in_maps.append({
            "d_idx": d_idx, "d_emb": embedding.astype(f32),
            "d_pf32": d_pf32, "d_pbf": d_pbf,
        })
    return in_maps


def _unpack(res_list):
    out = np.empty((B, TS, V), np.float32)
    for c in range(NCORES):
        do = res_list[c]["d_out"].astype(np.float32)  # [NM, P, TS, B] bf16
        out[:, :, c * VSH:(c + 1) * VSH] = \
            do.transpose(3, 2, 0, 1).reshape(B, TS, VP)[:, :, :VSH]
    return out


def kernel(tgt, h0, c0, enc_out, enc_mask, embedding, w_attn, w_ih0,
           w_ih_rest, w_hh, b_ih, b_hh, w_proj, b_proj, _trace=False):
    nsteps = int(os.environ.get("K_NSTEPS", TS))
    args = [np.asarray(a) for a in
            (tgt, h0, c0, enc_out, embedding, w_attn, w_ih0, w_ih_rest,
             w_hh, b_ih, b_hh, w_proj, b_proj)]
    in_maps = _prep_inputs(*args)
    if nsteps not in _prog_cache:
        _prog_cache[nsteps] = _build(nsteps)
    nc = _prog_cache[nsteps]
    res = run_bass_kernel_spmd(nc, in_maps, list(range(NCORES)), trace=_trace)
    kernel._last = res
    return _unpack(res.results)
